# revision 10
# baseline (speedup 1.0000x reference)
"""Bass/Trainium2 kernel for BiDirectionalCrossAttention (8-core SPMD).

Sharding: 8 cores = 4 batches x 2 head-groups (4 heads each).
Each core computes, for its (batch b, head-group g):
  - Q/K projections restricted to its 256 channels, channel-major [chan, token]
  - V projection in [token, chan] layout; ones-columns (softmax denominator
    rides along attn@V for free) are memset once instead of matmul'd
  - scoresT[kv, q] per head, exp on ScalarE, attn@V accumulation on PE
  - partial output projection Wout[:, cols_g] @ out_g  -> [512, 1024]
Host sums the two partials per batch and adds the folded bias
bout' = bout + Wout @ bv (V-bias commutes through softmax since rows sum to 1).

v2: fast softmax-denominator chain (reciprocal_approx_fast straight off
PSUM + packed DRAM-bounce broadcast), single-blob weight DMA, DMA issue
spread across engine DGE rings, PE pstate warmup, pipelined tail.
"""

import sys
import os

for _p in ("/opt/trn_rl_repo", "/root/.axon_site/_ro/trn_rl_repo"):
    if os.path.isdir(_p) and _p not in sys.path:
        sys.path.append(_p)

import numpy as np
import ml_dtypes

import concourse.bass as bass
import concourse.mybir as mybir
import concourse.tile as tile
from concourse.bass_utils import run_bass_kernel_spmd

BF16 = mybir.dt.bfloat16
F32 = mybir.dt.float32
F32R = mybir.dt.float32r
NP_BF16 = ml_dtypes.bfloat16

AF = mybir.ActivationFunctionType


def _split_multi_waits(nc: bass.Bass) -> None:
    """The walrus build here allows only one sync-wait per instruction.
    Tile attaches several; hoist the extras onto same-engine NOPs placed
    immediately before the instruction (same per-engine program order)."""
    uid = 0
    for f in nc.m.functions:
        for bb in f.blocks:
            insts = bb.instructions
            out = []
            changed = False
            for inst in insts:
                si = inst.sync_info
                if si is not None and si.on_wait is not None and len(si.on_wait) > 1:
                    waits = list(si.on_wait)
                    for w in waits[:-1]:
                        nop = mybir.InstNoOp(
                            name=f"splitwait-{uid}",
                            engine=inst.engine,
                            ins=[],
                            outs=[],
                            sync_info=mybir.SyncInfo(on_wait=[w], on_update=[]),
                        )
                        uid += 1
                        out.append(nop)
                    inst.sync_info = mybir.SyncInfo(
                        on_wait=[waits[-1]], on_update=list(si.on_update or [])
                    )
                    changed = True
                out.append(inst)
            if changed:
                bb.instructions = out
    return


def _build_program() -> bass.Bass:
    nc = bass.Bass()

    qx_d = nc.declare_dram_parameter("qx", [512, 1024], BF16, isOutput=False)
    kvx_d = nc.declare_dram_parameter("kvx", [512, 2048], BF16, isOutput=False)
    # weight blob: [128, 4, 1024] bf16; per k-chunk: wq 256 | wk 256 | wv 256 |
    # wo 256 (wo flattened [128, 1024] split across the 4 k-chunks)
    wb_d = nc.declare_dram_parameter("wb", [128, 4096], BF16, isOutput=False)
    bb_d = nc.declare_dram_parameter("bb", [128, 4], F32, isOutput=False)
    out_d = nc.declare_dram_parameter("out", [512, 1024], F32, isOutput=True)

    from contextlib import ExitStack

    with tile.TileContext(nc) as tc, ExitStack() as ctx:
        sb = ctx.enter_context(tc.tile_pool(name="sb", bufs=1))
        esb = ctx.enter_context(tc.tile_pool(name="esb", bufs=10))
        small = ctx.enter_context(tc.tile_pool(name="small", bufs=4))
        # PSUM budget (8 banks): "sc" 2 slots x [128,2,512] (2 banks) = 4,
        # "o" 4 slots x 1 bank = 4. Q/K-proj + out-proj borrow "o", V-proj "sc".
        sc_ps = ctx.enter_context(tc.tile_pool(name="scps", bufs=2, space="PSUM"))
        dpool = ctx.enter_context(tc.tile_pool(name="dram", bufs=2, space="DRAM"))
        o_ps = ctx.enter_context(tc.tile_pool(name="ops", bufs=4, space="PSUM"))

        # ---------------- SBUF tiles ----------------
        qx_s = sb.tile([128, 4, 1024], BF16, name="qx", tag="qx")
        kvx_s = sb.tile([128, 4, 2048], BF16, name="kvx", tag="kvx")
        wb_s = sb.tile([128, 4, 1024], BF16, name="wb", tag="wb")
        bb_s = sb.tile([128, 4], F32, name="bb", tag="bb")
        warm = sb.tile([1, 512], BF16, name="warm", tag="warm")
        qt_s = [sb.tile([128, 1024], BF16, name=f"qt{m}", tag=f"qt{m}") for m in range(2)]
        kt_s = [sb.tile([128, 2048], BF16, name=f"kt{m}", tag=f"kt{m}") for m in range(2)]
        v_s = sb.tile([128, 16, 4, 65], BF16, name="v", tag="v")
        ot_s = [sb.tile([128, 1024], BF16, name=f"ot{m}", tag=f"ot{m}") for m in range(2)]

        # weight views into the blob: per k-chunk columns
        def wq_ap(k, m):
            return wb_s[:, k, m * 128:(m + 1) * 128]

        def wk_ap(k, m):
            return wb_s[:, k, 256 + m * 128:256 + (m + 1) * 128]

        def wv_ap(k):
            return wb_s[:, k, 512:768]

        def wo_ap(m, mo):
            o = m * 512 + mo * 128
            return wb_s[:, o // 256, 768 + (o % 256):768 + (o % 256) + 128]

        bq_s = bb_s[:, 0:2]
        bk_s = bb_s[:, 2:4]

        # ---------------- warmup + constants ----------------
        nc.vector.memset(warm[:], 0.0)
        # ones columns of V (softmax denominator lanes), set once
        nc.vector.memset(v_s[:, :, :, 64:65], 1.0)

        # ---------------- input DMAs, spread across DGE rings ----------------
        def chunked(d, parts=128):
            return d.rearrange("(k p) n -> p k n", p=parts)

        nc.sync.dma_start(out=wb_s[:], in_=wb_d.rearrange("p (k n) -> p k n", k=4))
        nc.scalar.dma_start(out=bb_s[:], in_=bb_d[:])
        nc.scalar.dma_start(out=qx_s[:], in_=chunked(qx_d))
        nc.sync.dma_start(out=kvx_s[:, :, 0:512], in_=chunked(kvx_d[:, 0:512]))
        nc.gpsimd.dma_start(out=kvx_s[:, :, 512:2048],
                            in_=chunked(kvx_d[:, 512:2048]))

        # PE pstate warmup: harmless matmuls while DMAs land (pstate ramps
        # toward 2.4GHz only under continuous PE busy)
        for w in range(6):
            wps = o_ps.tile([128, 512], F32, name="o", tag="o", bufs=4)
            nc.tensor.matmul(wps, lhsT=warm[0:1, 0:128], rhs=warm[0:1, :],
                             start=True, stop=True)

        # ---------------- building blocks ----------------
        def qproj_group(m, t):
            ps = o_ps.tile([128, 512], F32, name="o", tag="o", bufs=4)
            for k in range(4):
                nc.tensor.matmul(
                    ps,
                    lhsT=wq_ap(k, m),
                    rhs=qx_s[:, k, t * 512:(t + 1) * 512],
                    start=(k == 0), stop=(k == 3),
                )
            nc.vector.tensor_scalar_add(
                out=qt_s[m][:, t * 512:(t + 1) * 512], in0=ps,
                scalar1=bq_s[:, m:m + 1],
            )

        def kproj_group(m, t):
            ps = o_ps.tile([128, 512], F32, name="o", tag="o", bufs=4)
            for k in range(4):
                nc.tensor.matmul(
                    ps,
                    lhsT=wk_ap(k, m),
                    rhs=kvx_s[:, k, t * 512:(t + 1) * 512],
                    start=(k == 0), stop=(k == 3),
                )
            nc.vector.tensor_scalar_add(
                out=kt_s[m][:, t * 512:(t + 1) * 512], in0=ps,
                scalar1=bk_s[:, m:m + 1],
            )

        def vproj_tile(tt):
            # [token, 256] -> strided copy into the 4x65 per-head layout
            ps = sc_ps.tile([128, 4, 64], F32, name="sc", tag="sc")
            for k in range(4):
                nc.tensor.matmul(
                    ps,
                    lhsT=kvx_s[:, k, tt * 128:(tt + 1) * 128],
                    rhs=wv_ap(k),
                    start=(k == 0), stop=(k == 3),
                )
            nc.vector.tensor_copy(out=v_s[:, tt, :, 0:64], in_=ps)

        o_tiles = {}
        recips = {}

        def norm_recip(m, t, tail=False):
            oA, oB = o_tiles[(m, t)]
            sd = dpool.tile([2, 512], F32, name="sd", tag="sd")
            if tail:
                # ScalarE is idle in the tail: 1/s = exp(-ln s); ln and exp
                # share an activation table so no table swap
                rc = small.tile([33, 512], F32, name="rc", tag="rc")
                nc.scalar.activation(out=rc[0:1, :], in_=oA[64:65, :], func=AF.Ln)
                nc.scalar.activation(out=rc[32:33, :], in_=oB[64:65, :], func=AF.Ln)
                rr = small.tile([33, 512], F32, name="rr", tag="rr")
                nc.scalar.activation(out=rr[0:1, :], in_=rc[0:1, :],
                                     func=AF.Exp, scale=-1.0)
                nc.scalar.activation(out=rr[32:33, :], in_=rc[32:33, :],
                                     func=AF.Exp, scale=-1.0)
            else:
                # both heads' sums -> one [33,512] reciprocal (DVE time scales
                # with free size; rows 1..31 are don't-care garbage)
                rc = small.tile([33, 512], F32, name="rc", tag="rc")
                nc.vector.tensor_copy(out=rc[0:1, :], in_=oA[64:65, :])
                nc.vector.tensor_copy(out=rc[32:33, :], in_=oB[64:65, :])
                rr = small.tile([33, 512], F32, name="rr", tag="rr")
                nc.vector.reciprocal(out=rr, in_=rc)
            # bounce 1/s through DRAM so it can be re-read with a 0-stride
            # (partition-broadcast) source AP
            nc.gpsimd.dma_start(out=sd[0:1, :], in_=rr[0:1, :])
            nc.gpsimd.dma_start(out=sd[1:2, :], in_=rr[32:33, :])
            recips[(m, t)] = sd

        bcs_tiles = {}

        def norm_bcast(m, t, engine=None):
            sd = recips.pop((m, t))
            # [128,512]: rows 0-63 <- 1/sA, rows 64-127 <- 1/sB (one DMA)
            bsrc = bass.AP(tensor=sd.tensor, offset=sd.offset,
                           ap=[[512, 2], [0, 64], [1, 512]])
            bcs = small.tile([128, 512], F32, name="bcs", tag="bcs", bufs=2)
            (engine or nc.gpsimd).dma_start(out=bcs, in_=bsrc)
            bcs_tiles[(m, t)] = bcs

        def norm_apply(m, t):
            qsl = slice(t * 512, (t + 1) * 512)
            oA, oB = o_tiles.pop((m, t))
            bcs = bcs_tiles.pop((m, t))
            nc.vector.tensor_mul(ot_s[m][0:64, qsl], oA[0:64, :], bcs[0:64, :])
            nc.vector.tensor_mul(ot_s[m][64:128, qsl], oB[0:64, :], bcs[64:128, :])

        fo_tiles = {}

        def outproj_group(t2, mo, engine="vector", dma=None):
            if t2 not in fo_tiles:
                fo_tiles[t2] = small.tile([128, 4, 512], F32, name="fo",
                                          tag="fo", bufs=2)
            fo = fo_tiles[t2]
            ps = o_ps.tile([128, 512], F32, name="o", tag="o", bufs=4)
            for m in range(2):
                nc.tensor.matmul(
                    ps,
                    lhsT=wo_ap(m, mo),
                    rhs=ot_s[m][:, t2 * 512:(t2 + 1) * 512],
                    start=(m == 0), stop=(m == 1),
                )
            if engine == "vector":
                nc.vector.tensor_copy(out=fo[:, mo, :], in_=ps)
            else:
                nc.scalar.activation(out=fo[:, mo, :], in_=ps, func=AF.Copy)
            (dma or nc.gpsimd).dma_start(
                out=out_d[mo * 128:(mo + 1) * 128, t2 * 512:(t2 + 1) * 512],
                in_=fo[:, mo, :],
            )

        # ---------------- pipelined schedule ----------------
        # 64 global iterations (4 units x 16 kv tiles); scores emitted one
        # iteration ahead so ScalarE's exp stream never waits on PE.
        units = [(0, 0), (1, 0), (0, 1), (1, 1)]
        iters = [(u, i) for u in units for i in range(16)]

        # interleave remaining projections + V tiles + norms + out-proj
        # into the per-iteration PE slack (ACT exp is the steady-state pacer)
        extra = {g: [] for g in range(64)}
        kplan = [(0, 1), (0, 2), (0, 3), (1, 0), (1, 1), (1, 2), (1, 3)]
        for idx, (m_, t_) in enumerate(kplan):
            extra[2 * idx + 1].append(lambda m_=m_, t_=t_: kproj_group(m_, t_))
        extra[0].append(lambda: qproj_group(1, 0))
        for tt in range(16):
            extra[tt].append(lambda tt=tt: vproj_tile(tt))
        post = {
            17: [lambda: norm_recip(0, 0)],
            18: [lambda: norm_bcast(0, 0)],
            20: [lambda: norm_apply(0, 0)],
            23: [lambda: qproj_group(0, 1)],
            25: [lambda: qproj_group(1, 1)],
            33: [lambda: norm_recip(1, 0)],
            34: [lambda: norm_bcast(1, 0)],
            36: [lambda: norm_apply(1, 0)],
            37: [lambda: outproj_group(0, 0)],
            39: [lambda: outproj_group(0, 1)],
            41: [lambda: outproj_group(0, 2)],
            43: [lambda: outproj_group(0, 3)],
            49: [lambda: norm_recip(0, 1)],
            50: [lambda: norm_bcast(0, 1)],
            52: [lambda: norm_apply(0, 1)],
        }

        qproj_group(0, 0)
        kproj_group(0, 0)

        sc_tiles = {}

        def emit_scores(g):
            (m, t), i = iters[g]
            ksl = slice(i * 128, (i + 1) * 128)
            qsl = slice(t * 512, (t + 1) * 512)
            sc = sc_ps.tile([128, 2, 512], F32, name="sc", tag="sc")
            nc.tensor.matmul(
                sc[:, 0, :], lhsT=kt_s[m][0:64, ksl], rhs=qt_s[m][0:64, qsl],
                start=True, stop=True, tile_position=(0, 0),
            )
            nc.tensor.matmul(
                sc[:, 1, :], lhsT=kt_s[m][64:128, ksl], rhs=qt_s[m][64:128, qsl],
                start=True, stop=True, tile_position=(64, 0),
            )
            sc_tiles[g] = sc

        emit_scores(0)
        for g in range(64):
            (m, t), i = iters[g]
            if g + 1 < 64:
                emit_scores(g + 1)
            sc = sc_tiles.pop(g)
            e = esb.tile([128, 2, 512], BF16, name="e", tag="e")
            nc.scalar.activation(out=e[:], in_=sc[:], func=AF.Exp, scale=0.125)
            for fn in extra.get(g, ()):
                fn()
            if i == 0:
                oA = o_ps.tile([65, 512], F32, name="o", tag="o", bufs=4)
                oB = o_ps.tile([65, 512], F32, name="o", tag="o", bufs=4)
                o_tiles[(m, t)] = (oA, oB)
            oA, oB = o_tiles[(m, t)]
            jA, jB = 2 * m, 2 * m + 1
            nc.tensor.matmul(
                oA, lhsT=v_s[:, i, jA, :], rhs=e[:, 0, :],
                start=(i == 0), stop=(i == 15),
            )
            nc.tensor.matmul(
                oB, lhsT=v_s[:, i, jB, :], rhs=e[:, 1, :],
                start=(i == 0), stop=(i == 15),
            )
            for fn in post.get(g, ()):
                fn()

        # ---------------- tail: last unit's norm + out-proj ----------------
        norm_recip(1, 1, tail=True)
        norm_bcast(1, 1, engine=nc.sync)
        norm_apply(1, 1)
        outproj_group(1, 0, engine="scalar", dma=nc.sync)
        outproj_group(1, 1, engine="vector", dma=nc.scalar)
        outproj_group(1, 2, engine="scalar", dma=nc.gpsimd)
        outproj_group(1, 3, engine="vector", dma=nc.sync)

    _split_multi_waits(nc)
    return nc


_PROGRAM = None


def _get_program() -> bass.Bass:
    global _PROGRAM
    if _PROGRAM is None:
        _PROGRAM = _build_program()
    return _PROGRAM


def _prep_core_inputs(c, q, kv, Wqkv, bqkv, Wout):
    b, g = c // 2, c % 2
    cs = slice(256 * g, 256 * g + 256)
    wq = Wqkv[cs, :].T  # [512, 256]
    wk = Wqkv[512 + 256 * g:512 + 256 * g + 256, :].T
    wv = Wqkv[1024 + 256 * g:1024 + 256 * g + 256, :].T
    wo = Wout[:, cs].T  # [256, 512]

    # pack weights: [128, 4, 1024] with per-k-chunk [wq 256|wk 256|wv 256|wo 256]
    wb = np.empty((128, 4, 1024), np.float32)
    for k in range(4):
        rs = slice(128 * k, 128 * (k + 1))
        wb[:, k, 0:256] = wq[rs, :]
        wb[:, k, 256:512] = wk[rs, :]
        wb[:, k, 512:768] = wv[rs, :]
        wb[:, k, 768:1024] = wo.reshape(2, 128, 512).transpose(1, 0, 2).reshape(
            128, 1024)[:, 256 * k:256 * (k + 1)]
    bb = np.empty((128, 4), np.float32)
    bb[:, 0:2] = bqkv[cs].reshape(2, 128).T
    bb[:, 2:4] = bqkv[512 + 256 * g:512 + 256 * g + 256].reshape(2, 128).T
    return {
        "qx": np.ascontiguousarray(q[b].reshape(512, 1024)).astype(NP_BF16),
        "kvx": np.ascontiguousarray(kv[b].reshape(512, 2048)).astype(NP_BF16),
        "wb": np.ascontiguousarray(wb.reshape(128, 4096)).astype(NP_BF16),
        "bb": np.ascontiguousarray(bb),
    }


def kernel(q, kv, Wqkv, bqkv, Wout, bout):
    q = np.asarray(q, np.float32)
    kv = np.asarray(kv, np.float32)
    Wqkv = np.asarray(Wqkv, np.float32)
    bqkv = np.asarray(bqkv, np.float32)
    Wout = np.asarray(Wout, np.float32)
    bout = np.asarray(bout, np.float32)

    nc = _get_program()
    in_maps = [_prep_core_inputs(c, q, kv, Wqkv, bqkv, Wout) for c in range(8)]
    res = run_bass_kernel_spmd(nc, in_maps, list(range(8))).results

    # V-bias folds through softmax (rows sum to 1): bout' = bout + Wout @ bv
    bout_adj = bout + Wout @ bqkv[1024:1536]
    out = np.empty((4, 512, 32, 32), np.float32)
    for b in range(4):
        o = res[2 * b]["out"] + res[2 * b + 1]["out"] + bout_adj[:, None]
        out[b] = o.reshape(512, 32, 32)
    return out


# revision 23
# speedup vs baseline: 1.3373x; 1.3373x over previous
"""Bass/Trainium2 kernel for BiDirectionalCrossAttention (8-core SPMD).

Sharding: 8 cores = 4 batches x 2 head-groups (4 heads each).
Each core computes, for its (batch b, head-group g):
  - Q/K projections restricted to its 256 channels, channel-major [chan, token]
  - V projection in [token, chan] layout; ones-columns (softmax denominator
    rides along attn@V for free) are memset once instead of matmul'd
  - scoresT[kv, q] per head, exp on ScalarE, attn@V accumulation on PE
  - partial output projection Wout[:, cols_g] @ out_g  -> [512, 1024]
Host sums the two partials per batch and adds the folded bias
bout' = bout + Wout @ bv (V-bias commutes through softmax since rows sum to 1).

v2: fast softmax-denominator chain (reciprocal_approx_fast straight off
PSUM + packed DRAM-bounce broadcast), single-blob weight DMA, DMA issue
spread across engine DGE rings, PE pstate warmup, pipelined tail.
"""

import sys
import os

for _p in ("/opt/trn_rl_repo", "/root/.axon_site/_ro/trn_rl_repo"):
    if os.path.isdir(_p) and _p not in sys.path:
        sys.path.append(_p)

import numpy as np
import ml_dtypes

import concourse.bass as bass
import concourse.mybir as mybir
import concourse.tile as tile
from concourse.bass_utils import run_bass_kernel_spmd

BF16 = mybir.dt.bfloat16
F32 = mybir.dt.float32
F32R = mybir.dt.float32r
NP_BF16 = ml_dtypes.bfloat16

AF = mybir.ActivationFunctionType


def _split_multi_waits(nc: bass.Bass) -> None:
    """The walrus build here allows only one sync-wait per instruction.
    Tile attaches several; hoist the extras onto same-engine NOPs placed
    immediately before the instruction (same per-engine program order)."""
    uid = 0
    for f in nc.m.functions:
        for bb in f.blocks:
            insts = bb.instructions
            out = []
            changed = False
            for inst in insts:
                si = inst.sync_info
                if si is not None and si.on_wait is not None and len(si.on_wait) > 1:
                    waits = list(si.on_wait)
                    for w in waits[:-1]:
                        nop = mybir.InstNoOp(
                            name=f"splitwait-{uid}",
                            engine=inst.engine,
                            ins=[],
                            outs=[],
                            sync_info=mybir.SyncInfo(on_wait=[w], on_update=[]),
                        )
                        uid += 1
                        out.append(nop)
                    inst.sync_info = mybir.SyncInfo(
                        on_wait=[waits[-1]], on_update=list(si.on_update or [])
                    )
                    changed = True
                out.append(inst)
            if changed:
                bb.instructions = out
    return


def _build_program() -> bass.Bass:
    nc = bass.Bass()

    qx_d = nc.declare_dram_parameter("qx", [512, 1024], BF16, isOutput=False)
    kvx_d = nc.declare_dram_parameter("kvx", [512, 2048], BF16, isOutput=False)
    # weight blob: [128, 4, 1024] bf16; per k-chunk: wq 256 | wk 256 | wv 256 |
    # wo 256 (wo flattened [128, 1024] split across the 4 k-chunks)
    wb_d = nc.declare_dram_parameter("wb", [128, 4096], BF16, isOutput=False)
    bb_d = nc.declare_dram_parameter("bb", [128, 4], F32, isOutput=False)
    out_d = nc.declare_dram_parameter("out", [512, 1024], F32, isOutput=True)

    from contextlib import ExitStack

    with tile.TileContext(nc) as tc, ExitStack() as ctx:
        sb = ctx.enter_context(tc.tile_pool(name="sb", bufs=1))
        esb = ctx.enter_context(tc.tile_pool(name="esb", bufs=10))
        small = ctx.enter_context(tc.tile_pool(name="small", bufs=4))
        # PSUM budget (8 banks): "sc" 2 slots x [128,2,512] (2 banks) = 4,
        # "o" 4 slots x 1 bank = 4. Q/K-proj + out-proj borrow "o", V-proj "sc".
        sc_ps = ctx.enter_context(tc.tile_pool(name="scps", bufs=2, space="PSUM"))
        dpool = ctx.enter_context(tc.tile_pool(name="dram", bufs=2, space="DRAM"))
        o_ps = ctx.enter_context(tc.tile_pool(name="ops", bufs=4, space="PSUM"))

        # ---------------- SBUF tiles ----------------
        qx_s = sb.tile([128, 4, 1024], BF16, name="qx", tag="qx")
        kvx_s = sb.tile([128, 4, 2048], BF16, name="kvx", tag="kvx")
        wb_s = sb.tile([128, 4, 1024], BF16, name="wb", tag="wb")
        bb_s = sb.tile([128, 4], F32, name="bb", tag="bb")
        warm = sb.tile([1, 512], BF16, name="warm", tag="warm")
        # 0/1 selector for the PE partition-broadcast of 1/s (fp32 matmul):
        # bc[p,:] = rr[0,:] for p<64 else rr[32,:]; rows 1..31 are zero so
        # rr's don't-care rows contribute nothing
        sel_s = sb.tile([33, 128], F32, name="sel", tag="sel")
        rc_s = sb.tile([33, 512], F32, name="rc", tag="rc")
        rr_s = sb.tile([33, 512], F32, name="rrs", tag="rrs")
        qt_s = [sb.tile([128, 1024], BF16, name=f"qt{m}", tag=f"qt{m}") for m in range(2)]
        kt_s = [sb.tile([128, 2048], BF16, name=f"kt{m}", tag=f"kt{m}") for m in range(2)]
        v_s = sb.tile([128, 16, 4, 65], BF16, name="v", tag="v")
        ot_s = [sb.tile([128, 1024], BF16, name=f"ot{m}", tag=f"ot{m}") for m in range(2)]

        # weight views into the blob: per k-chunk columns
        def wq_ap(k, m):
            return wb_s[:, k, m * 128:(m + 1) * 128]

        def wk_ap(k, m):
            return wb_s[:, k, 256 + m * 128:256 + (m + 1) * 128]

        def wv_ap(k):
            return wb_s[:, k, 512:768]

        def wo_ap(m, mo):
            o = m * 512 + mo * 128
            return wb_s[:, o // 256, 768 + (o % 256):768 + (o % 256) + 128]

        bq_s = bb_s[:, 0:2]
        bk_s = bb_s[:, 2:4]

        # ---------------- warmup + constants ----------------
        nc.vector.memset(warm[:], 0.0)
        # ones columns of V (softmax denominator lanes), set once
        nc.vector.memset(v_s[:, :, :, 64:65], 1.0)
        nc.vector.memset(sel_s[:], 0.0)
        nc.vector.memset(sel_s[0:1, 0:64], 1.0)
        nc.vector.memset(sel_s[32:33, 64:128], 1.0)
        # rc rows 1..31 must stay finite (1.0) so 1/rc is NaN-free
        nc.vector.memset(rc_s[:], 1.0)

        # ---------------- input DMAs, spread across DGE rings ----------------
        def chunked(d, parts=128):
            return d.rearrange("(k p) n -> p k n", p=parts)

        # fine-grained, consumption-ordered, spread across the three DGE rings
        wbv = wb_d.rearrange("p (k n) -> p k n", k=4)
        nc.sync.dma_start(out=wb_s[:, :, 0:256], in_=wbv[:, :, 0:256])      # wq
        nc.scalar.dma_start(out=bb_s[:], in_=bb_d[:])
        nc.scalar.dma_start(out=qx_s[:, :, 0:512], in_=chunked(qx_d[:, 0:512]))
        nc.sync.dma_start(out=wb_s[:, :, 256:512], in_=wbv[:, :, 256:512])  # wk
        nc.gpsimd.dma_start(out=wb_s[:, :, 512:768], in_=wbv[:, :, 512:768])  # wv
        nc.sync.dma_start(out=kvx_s[:, :, 0:512], in_=chunked(kvx_d[:, 0:512]))
        nc.scalar.dma_start(out=qx_s[:, :, 512:1024],
                            in_=chunked(qx_d[:, 512:1024]))
        nc.sync.dma_start(out=kvx_s[:, :, 512:1024],
                          in_=chunked(kvx_d[:, 512:1024]))
        nc.gpsimd.dma_start(out=kvx_s[:, :, 1024:1536],
                            in_=chunked(kvx_d[:, 1024:1536]))
        nc.gpsimd.dma_start(out=kvx_s[:, :, 1536:2048],
                            in_=chunked(kvx_d[:, 1536:2048]))
        nc.gpsimd.dma_start(out=wb_s[:, :, 768:1024], in_=wbv[:, :, 768:1024])  # wo

        # PE pstate warmup: harmless matmuls while DMAs land (pstate ramps
        # toward 2.4GHz only under continuous PE busy)
        for w in range(6):
            wps = o_ps.tile([128, 512], F32, name="o", tag="o", bufs=4)
            nc.tensor.matmul(wps, lhsT=warm[0:1, 0:128], rhs=warm[0:1, :],
                             start=True, stop=True)

        # ---------------- building blocks ----------------
        def qproj_group(m, t):
            ps = o_ps.tile([128, 512], F32, name="o", tag="o", bufs=4)
            for k in range(4):
                nc.tensor.matmul(
                    ps,
                    lhsT=wq_ap(k, m),
                    rhs=qx_s[:, k, t * 512:(t + 1) * 512],
                    start=(k == 0), stop=(k == 3),
                )
            nc.vector.tensor_scalar_add(
                out=qt_s[m][:, t * 512:(t + 1) * 512], in0=ps,
                scalar1=bq_s[:, m:m + 1],
            )

        def kproj_group(m, t):
            ps = o_ps.tile([128, 512], F32, name="o", tag="o", bufs=4)
            for k in range(4):
                nc.tensor.matmul(
                    ps,
                    lhsT=wk_ap(k, m),
                    rhs=kvx_s[:, k, t * 512:(t + 1) * 512],
                    start=(k == 0), stop=(k == 3),
                )
            nc.vector.tensor_scalar_add(
                out=kt_s[m][:, t * 512:(t + 1) * 512], in0=ps,
                scalar1=bk_s[:, m:m + 1],
            )

        def vproj_tile(tt):
            # [token, 256] -> strided copy into the 4x65 per-head layout
            ps = sc_ps.tile([128, 4, 64], F32, name="sc", tag="sc")
            for k in range(4):
                nc.tensor.matmul(
                    ps,
                    lhsT=kvx_s[:, k, tt * 128:(tt + 1) * 128],
                    rhs=wv_ap(k),
                    start=(k == 0), stop=(k == 3),
                )
            nc.vector.tensor_copy(out=v_s[:, tt, :, 0:64], in_=ps)

        o_tiles = {}
        recips = {}

        def norm_recip(m, t, tail=False):
            oA, oB = o_tiles[(m, t)]
            if tail:
                # ScalarE is idle in the tail: 1/s = exp(-ln s); ln and exp
                # share an activation table so no table swap
                nc.scalar.activation(out=rc_s[0:1, :], in_=oA[64:65, :], func=AF.Ln)
                nc.scalar.activation(out=rc_s[32:33, :], in_=oB[64:65, :], func=AF.Ln)
                nc.scalar.activation(out=rr_s[0:1, :], in_=rc_s[0:1, :],
                                     func=AF.Exp, scale=-1.0)
                nc.scalar.activation(out=rr_s[32:33, :], in_=rc_s[32:33, :],
                                     func=AF.Exp, scale=-1.0)
            else:
                # both heads' sums -> one [33,512] reciprocal (DVE time scales
                # with free size; rows 1..31 are 1.0 so 1/rc stays finite)
                nc.vector.tensor_copy(out=rc_s[0:1, :], in_=oA[64:65, :])
                nc.vector.tensor_copy(out=rc_s[32:33, :], in_=oB[64:65, :])
                nc.vector.reciprocal(out=rr_s[:], in_=rc_s[:])

        bcs_tiles = {}

        def norm_bcast(m, t):
            # partition-broadcast 1/s on the PE: fp32 selector matmul, then
            # stage to SBUF (DVE reads at most one PSUM operand per op)
            bc = o_ps.tile([128, 512], F32, name="o", tag="o", bufs=4)
            nc.tensor.matmul(bc, lhsT=sel_s[:], rhs=rr_s[:], start=True, stop=True)
            bcs = small.tile([128, 512], F32, name="bcs", tag="bcs", bufs=2)
            nc.vector.tensor_copy(out=bcs, in_=bc)
            bcs_tiles[(m, t)] = bcs

        def norm_apply(m, t):
            qsl = slice(t * 512, (t + 1) * 512)
            oA, oB = o_tiles.pop((m, t))
            bcs = bcs_tiles.pop((m, t))
            nc.vector.tensor_mul(ot_s[m][0:64, qsl], oA[0:64, :], bcs[0:64, :])
            nc.vector.tensor_mul(ot_s[m][64:128, qsl], oB[0:64, :], bcs[64:128, :])

        fo_tiles = {}

        def outproj_group(t2, mo, engine="vector", dma=None):
            if t2 not in fo_tiles:
                fo_tiles[t2] = small.tile([128, 4, 512], F32, name="fo",
                                          tag="fo", bufs=2)
            fo = fo_tiles[t2]
            ps = o_ps.tile([128, 512], F32, name="o", tag="o", bufs=4)
            for m in range(2):
                nc.tensor.matmul(
                    ps,
                    lhsT=wo_ap(m, mo),
                    rhs=ot_s[m][:, t2 * 512:(t2 + 1) * 512],
                    start=(m == 0), stop=(m == 1),
                )
            if engine == "vector":
                nc.vector.tensor_copy(out=fo[:, mo, :], in_=ps)
            else:
                nc.scalar.activation(out=fo[:, mo, :], in_=ps, func=AF.Copy)
            (dma or nc.gpsimd).dma_start(
                out=out_d[mo * 128:(mo + 1) * 128, t2 * 512:(t2 + 1) * 512],
                in_=fo[:, mo, :],
            )

        # ---------------- pipelined schedule ----------------
        # 64 global iterations (4 units x 16 kv tiles); scores emitted one
        # iteration ahead so ScalarE's exp stream never waits on PE.
        units = [(0, 0), (1, 0), (0, 1), (1, 1)]
        iters = [(u, i) for u in units for i in range(16)]

        # interleave remaining projections + V tiles + norms + out-proj
        # into the per-iteration PE slack (ACT exp is the steady-state pacer)
        extra = {g: [] for g in range(64)}
        kplan = [(0, 1), (0, 2), (0, 3), (1, 0), (1, 1), (1, 2), (1, 3)]
        for idx, (m_, t_) in enumerate(kplan):
            extra[2 * idx + 1].append(lambda m_=m_, t_=t_: kproj_group(m_, t_))
        extra[0].append(lambda: qproj_group(1, 0))
        for tt in range(16):
            extra[tt].append(lambda tt=tt: vproj_tile(tt))
        post = {
            17: [lambda: norm_recip(0, 0)],
            21: [lambda: norm_bcast(0, 0)],
            22: [lambda: norm_apply(0, 0)],
            25: [lambda: qproj_group(0, 1)],
            27: [lambda: qproj_group(1, 1)],
            33: [lambda: norm_recip(1, 0)],
            37: [lambda: norm_bcast(1, 0)],
            38: [lambda: norm_apply(1, 0)],
            41: [lambda: outproj_group(0, 0)],
            43: [lambda: outproj_group(0, 1)],
            45: [lambda: outproj_group(0, 2)],
            47: [lambda: outproj_group(0, 3)],
            49: [lambda: norm_recip(0, 1)],
            53: [lambda: norm_bcast(0, 1)],
            54: [lambda: norm_apply(0, 1)],
        }

        qproj_group(0, 0)
        kproj_group(0, 0)

        sc_tiles = {}

        def emit_scores(g):
            (m, t), i = iters[g]
            ksl = slice(i * 128, (i + 1) * 128)
            qsl = slice(t * 512, (t + 1) * 512)
            sc = sc_ps.tile([128, 2, 512], F32, name="sc", tag="sc")
            nc.tensor.matmul(
                sc[:, 0, :], lhsT=kt_s[m][0:64, ksl], rhs=qt_s[m][0:64, qsl],
                start=True, stop=True, tile_position=(0, 0),
            )
            nc.tensor.matmul(
                sc[:, 1, :], lhsT=kt_s[m][64:128, ksl], rhs=qt_s[m][64:128, qsl],
                start=True, stop=True, tile_position=(64, 0),
            )
            sc_tiles[g] = sc

        emit_scores(0)
        e_tiles = {}
        for g in range(64):
            (m, t), i = iters[g]
            u = g // 16
            if g + 1 < 64:
                emit_scores(g + 1)
            sc = sc_tiles.pop(g)
            e = esb.tile([128, 2, 512], BF16, name="e", tag="e", bufs=12)
            nc.scalar.activation(out=e[:], in_=sc[:], func=AF.Exp, scale=0.125)
            e_tiles[i] = e
            for fn in extra.get(g, ()):
                fn()
            # attn@V: unit 0 accumulates in step; later units defer their
            # first 8 kv tiles (so the previous unit's normalization can
            # release its PSUM accumulators) and catch up two tiles/iter.
            # ScalarE never starves: scores for g+1 are emitted above.
            if u == 0:
                todo = [i]
            elif i < 9:
                todo = []
            elif i < 15:
                todo = [2 * (i - 9), 2 * (i - 9) + 1]
            else:
                todo = [12, 13, 14, 15]
            if todo and (m, t) not in o_tiles:
                oA = o_ps.tile([65, 512], F32, name="o", tag="o", bufs=4)
                oB = o_ps.tile([65, 512], F32, name="o", tag="o", bufs=4)
                o_tiles[(m, t)] = (oA, oB)
            if todo:
                oA, oB = o_tiles[(m, t)]
                jA, jB = 2 * m, 2 * m + 1
                for j in todo:
                    ej = e_tiles[j]
                    nc.tensor.matmul(
                        oA, lhsT=v_s[:, j, jA, :], rhs=ej[:, 0, :],
                        start=(j == 0), stop=(j == 15),
                    )
                    nc.tensor.matmul(
                        oB, lhsT=v_s[:, j, jB, :], rhs=ej[:, 1, :],
                        start=(j == 0), stop=(j == 15),
                    )
            for fn in post.get(g, ()):
                fn()

        # ---------------- tail: last unit's norm + out-proj ----------------
        norm_recip(1, 1, tail=True)
        norm_bcast(1, 1)
        norm_apply(1, 1)
        outproj_group(1, 0, engine="scalar", dma=nc.sync)
        outproj_group(1, 1, engine="vector", dma=nc.scalar)
        outproj_group(1, 2, engine="scalar", dma=nc.gpsimd)
        outproj_group(1, 3, engine="vector", dma=nc.sync)

    _split_multi_waits(nc)
    return nc


_PROGRAM = None


def _get_program() -> bass.Bass:
    global _PROGRAM
    if _PROGRAM is None:
        _PROGRAM = _build_program()
    return _PROGRAM


def _prep_core_inputs(c, q, kv, Wqkv, bqkv, Wout):
    b, g = c // 2, c % 2
    cs = slice(256 * g, 256 * g + 256)
    wq = Wqkv[cs, :].T  # [512, 256]
    wk = Wqkv[512 + 256 * g:512 + 256 * g + 256, :].T
    wv = Wqkv[1024 + 256 * g:1024 + 256 * g + 256, :].T
    wo = Wout[:, cs].T  # [256, 512]

    # pack weights: [128, 4, 1024] with per-k-chunk [wq 256|wk 256|wv 256|wo 256]
    wb = np.empty((128, 4, 1024), np.float32)
    for k in range(4):
        rs = slice(128 * k, 128 * (k + 1))
        wb[:, k, 0:256] = wq[rs, :]
        wb[:, k, 256:512] = wk[rs, :]
        wb[:, k, 512:768] = wv[rs, :]
        wb[:, k, 768:1024] = wo.reshape(2, 128, 512).transpose(1, 0, 2).reshape(
            128, 1024)[:, 256 * k:256 * (k + 1)]
    bb = np.empty((128, 4), np.float32)
    bb[:, 0:2] = bqkv[cs].reshape(2, 128).T
    bb[:, 2:4] = bqkv[512 + 256 * g:512 + 256 * g + 256].reshape(2, 128).T
    return {
        "qx": np.ascontiguousarray(q[b].reshape(512, 1024)).astype(NP_BF16),
        "kvx": np.ascontiguousarray(kv[b].reshape(512, 2048)).astype(NP_BF16),
        "wb": np.ascontiguousarray(wb.reshape(128, 4096)).astype(NP_BF16),
        "bb": np.ascontiguousarray(bb),
    }


def kernel(q, kv, Wqkv, bqkv, Wout, bout):
    q = np.asarray(q, np.float32)
    kv = np.asarray(kv, np.float32)
    Wqkv = np.asarray(Wqkv, np.float32)
    bqkv = np.asarray(bqkv, np.float32)
    Wout = np.asarray(Wout, np.float32)
    bout = np.asarray(bout, np.float32)

    nc = _get_program()
    in_maps = [_prep_core_inputs(c, q, kv, Wqkv, bqkv, Wout) for c in range(8)]
    res = run_bass_kernel_spmd(nc, in_maps, list(range(8))).results

    # V-bias folds through softmax (rows sum to 1): bout' = bout + Wout @ bv
    bout_adj = bout + Wout @ bqkv[1024:1536]
    out = np.empty((4, 512, 32, 32), np.float32)
    for b in range(4):
        o = res[2 * b]["out"] + res[2 * b + 1]["out"] + bout_adj[:, None]
        out[b] = o.reshape(512, 32, 32)
    return out


# revision 29
# speedup vs baseline: 1.4734x; 1.1018x over previous
"""Bass/Trainium2 kernel for BiDirectionalCrossAttention (8-core SPMD).

Sharding: 8 cores = 4 batches x 2 head-groups (4 heads each).
Each core computes, for its (batch b, head-group g):
  - Q/K projections restricted to its 256 channels, channel-major [chan, token]
  - V projection in [token, chan] layout; ones-columns (softmax denominator
    rides along attn@V for free) are memset once instead of matmul'd
  - scoresT[kv, q] per head, exp on ScalarE, attn@V accumulation on PE
  - partial output projection Wout[:, cols_g] @ out_g  -> [512, 1024]
Host sums the two partials per batch and adds the folded bias
bout' = bout + Wout @ bv (V-bias commutes through softmax since rows sum to 1).

v2: fast softmax-denominator chain (reciprocal_approx_fast straight off
PSUM + packed DRAM-bounce broadcast), single-blob weight DMA, DMA issue
spread across engine DGE rings, PE pstate warmup, pipelined tail.
"""

import sys
import os

for _p in ("/opt/trn_rl_repo", "/root/.axon_site/_ro/trn_rl_repo"):
    if os.path.isdir(_p) and _p not in sys.path:
        sys.path.append(_p)

import numpy as np
import ml_dtypes

import concourse.bass as bass
import concourse.mybir as mybir
import concourse.tile as tile
from concourse.bass_utils import run_bass_kernel_spmd

BF16 = mybir.dt.bfloat16
F32 = mybir.dt.float32
F32R = mybir.dt.float32r
NP_BF16 = ml_dtypes.bfloat16

AF = mybir.ActivationFunctionType


def _split_multi_waits(nc: bass.Bass) -> None:
    """The walrus build here allows only one sync-wait per instruction.
    Tile attaches several; hoist the extras onto same-engine NOPs placed
    immediately before the instruction (same per-engine program order)."""
    uid = 0
    for f in nc.m.functions:
        for bb in f.blocks:
            insts = bb.instructions
            out = []
            changed = False
            for inst in insts:
                si = inst.sync_info
                if si is not None and si.on_wait is not None and len(si.on_wait) > 1:
                    waits = list(si.on_wait)
                    for w in waits[:-1]:
                        nop = mybir.InstNoOp(
                            name=f"splitwait-{uid}",
                            engine=inst.engine,
                            ins=[],
                            outs=[],
                            sync_info=mybir.SyncInfo(on_wait=[w], on_update=[]),
                        )
                        uid += 1
                        out.append(nop)
                    inst.sync_info = mybir.SyncInfo(
                        on_wait=[waits[-1]], on_update=list(si.on_update or [])
                    )
                    changed = True
                out.append(inst)
            if changed:
                bb.instructions = out
    return


def _build_program() -> bass.Bass:
    nc = bass.Bass()

    qx_d = nc.declare_dram_parameter("qx", [512, 1024], BF16, isOutput=False)
    kvx_d = nc.declare_dram_parameter("kvx", [512, 2048], BF16, isOutput=False)
    # weight blob: [128, 4, 1024] bf16; per k-chunk: wq 256 | wk 256 | wv 256 |
    # wo 256 (wo flattened [128, 1024] split across the 4 k-chunks)
    wb_d = nc.declare_dram_parameter("wb", [128, 4096], BF16, isOutput=False)
    bb_d = nc.declare_dram_parameter("bb", [128, 4], F32, isOutput=False)
    out_d = nc.declare_dram_parameter("out", [512, 1024], F32, isOutput=True)

    from contextlib import ExitStack

    with tile.TileContext(nc) as tc, ExitStack() as ctx:
        sb = ctx.enter_context(tc.tile_pool(name="sb", bufs=1))
        esb = ctx.enter_context(tc.tile_pool(name="esb", bufs=10))
        small = ctx.enter_context(tc.tile_pool(name="small", bufs=4))
        # PSUM budget (8 banks): "sc" 2 slots x [128,2,512] (2 banks) = 4,
        # "o" 4 slots x 1 bank = 4. Q/K-proj + out-proj borrow "o", V-proj "sc".
        sc_ps = ctx.enter_context(tc.tile_pool(name="scps", bufs=2, space="PSUM"))
        dpool = ctx.enter_context(tc.tile_pool(name="dram", bufs=2, space="DRAM"))
        o_ps = ctx.enter_context(tc.tile_pool(name="ops", bufs=4, space="PSUM"))

        # ---------------- SBUF tiles ----------------
        qx_s = sb.tile([128, 4, 1024], BF16, name="qx", tag="qx")
        kvx_s = sb.tile([128, 4, 2048], BF16, name="kvx", tag="kvx")
        wb_s = sb.tile([128, 4, 1024], BF16, name="wb", tag="wb")
        bb_s = sb.tile([128, 4], F32, name="bb", tag="bb")
        warm = sb.tile([1, 512], BF16, name="warm", tag="warm")
        # 0/1 selector for the PE partition-broadcast of 1/s (fp32 matmul):
        # bc[p,:] = rr[0,:] for p<64 else rr[32,:]; rows 1..31 are zero so
        # rr's don't-care rows contribute nothing
        sel_s = sb.tile([33, 128], F32R, name="sel", tag="sel")
        self32 = sb.tile([33, 128], F32, name="self32", tag="self32")
        rc_s = sb.tile([33, 512], F32, name="rc", tag="rc")
        rr_s = sb.tile([33, 512], F32R, name="rrs", tag="rrs")
        qt_s = [sb.tile([128, 1024], BF16, name=f"qt{m}", tag=f"qt{m}") for m in range(2)]
        kt_s = [sb.tile([128, 2048], BF16, name=f"kt{m}", tag=f"kt{m}") for m in range(2)]
        v_s = sb.tile([128, 16, 4, 65], BF16, name="v", tag="v")
        ot_s = [sb.tile([128, 1024], BF16, name=f"ot{m}", tag=f"ot{m}") for m in range(2)]

        # weight views into the blob: per k-chunk columns
        def wq_ap(k, m):
            return wb_s[:, k, m * 128:(m + 1) * 128]

        def wk_ap(k, m):
            return wb_s[:, k, 256 + m * 128:256 + (m + 1) * 128]

        def wv_ap(k):
            return wb_s[:, k, 512:768]

        def wo_ap(m, mo):
            o = m * 512 + mo * 128
            return wb_s[:, o // 256, 768 + (o % 256):768 + (o % 256) + 128]

        bq_s = bb_s[:, 0:2]
        bk_s = bb_s[:, 2:4]

        # ---------------- warmup + constants ----------------
        nc.vector.memset(warm[:], 0.0)
        # ones columns of V (softmax denominator lanes), set once
        nc.vector.memset(v_s[:, :, :, 64:65], 1.0)
        nc.vector.memset(self32[:], 0.0)
        nc.vector.memset(self32[0:1, 0:64], 1.0)
        nc.vector.memset(self32[32:33, 64:128], 1.0)
        nc.vector.tensor_copy(out=sel_s[:], in_=self32[:])
        # rc rows 1..31 must stay finite (1.0) so 1/rc is NaN-free
        nc.vector.memset(rc_s[:], 1.0)

        # ---------------- input DMAs, spread across DGE rings ----------------
        def chunked(d, parts=128):
            return d.rearrange("(k p) n -> p k n", p=parts)

        # consumption-ordered on ONE ring: concurrent rings fair-share the
        # DMA fabric and starve the critical stream (measured)
        wbv = wb_d.rearrange("p (k n) -> p k n", k=4)
        nc.scalar.dma_start(out=bb_s[:], in_=bb_d[:])
        nc.sync.dma_start(out=wb_s[:, :, 0:256], in_=wbv[:, :, 0:256])      # wq
        nc.sync.dma_start(out=qx_s[:, :, 0:512], in_=chunked(qx_d[:, 0:512]))
        nc.sync.dma_start(out=wb_s[:, :, 256:512], in_=wbv[:, :, 256:512])  # wk
        nc.sync.dma_start(out=kvx_s[:, :, 0:512], in_=chunked(kvx_d[:, 0:512]))
        nc.sync.dma_start(out=qx_s[:, :, 512:1024],
                          in_=chunked(qx_d[:, 512:1024]))
        nc.sync.dma_start(out=wb_s[:, :, 512:768], in_=wbv[:, :, 512:768])  # wv
        nc.sync.dma_start(out=kvx_s[:, :, 512:1024],
                          in_=chunked(kvx_d[:, 512:1024]))
        nc.sync.dma_start(out=kvx_s[:, :, 1024:1536],
                          in_=chunked(kvx_d[:, 1024:1536]))
        nc.sync.dma_start(out=kvx_s[:, :, 1536:2048],
                          in_=chunked(kvx_d[:, 1536:2048]))
        nc.sync.dma_start(out=wb_s[:, :, 768:1024], in_=wbv[:, :, 768:1024])  # wo

        # PE pstate warmup: harmless matmuls while DMAs land (pstate ramps
        # toward 2.4GHz only under continuous PE busy)
        for w in range(6):
            wps = o_ps.tile([128, 512], F32, name="o", tag="o", bufs=4)
            nc.tensor.matmul(wps, lhsT=warm[0:1, 0:128], rhs=warm[0:1, :],
                             start=True, stop=True)

        # ---------------- building blocks ----------------
        def qproj_group(m, t):
            ps = o_ps.tile([128, 512], F32, name="o", tag="o", bufs=4)
            for k in range(4):
                nc.tensor.matmul(
                    ps,
                    lhsT=wq_ap(k, m),
                    rhs=qx_s[:, k, t * 512:(t + 1) * 512],
                    start=(k == 0), stop=(k == 3),
                )
            nc.vector.tensor_scalar_add(
                out=qt_s[m][:, t * 512:(t + 1) * 512], in0=ps,
                scalar1=bq_s[:, m:m + 1],
            )

        def kproj_group(m, t):
            ps = o_ps.tile([128, 512], F32, name="o", tag="o", bufs=4)
            for k in range(4):
                nc.tensor.matmul(
                    ps,
                    lhsT=wk_ap(k, m),
                    rhs=kvx_s[:, k, t * 512:(t + 1) * 512],
                    start=(k == 0), stop=(k == 3),
                )
            nc.vector.tensor_scalar_add(
                out=kt_s[m][:, t * 512:(t + 1) * 512], in0=ps,
                scalar1=bk_s[:, m:m + 1],
            )

        def vproj_tile(tt):
            # [token, 256] -> strided copy into the 4x65 per-head layout
            ps = sc_ps.tile([128, 4, 64], F32, name="sc", tag="sc")
            for k in range(4):
                nc.tensor.matmul(
                    ps,
                    lhsT=kvx_s[:, k, tt * 128:(tt + 1) * 128],
                    rhs=wv_ap(k),
                    start=(k == 0), stop=(k == 3),
                )
            nc.vector.tensor_copy(out=v_s[:, tt, :, 0:64], in_=ps)

        o_tiles = {}
        recips = {}

        def norm_recip(m, t, tail=False):
            oA, oB = o_tiles[(m, t)]
            # 1/s = exp(-ln s) on ScalarE: ln and exp share an activation
            # table (no swap); the DVE reciprocal is 3.3us and stalls the PE
            nc.scalar.activation(out=rc_s[0:1, :], in_=oA[64:65, :], func=AF.Ln)
            nc.scalar.activation(out=rc_s[32:33, :], in_=oB[64:65, :], func=AF.Ln)
            nc.scalar.activation(out=rr_s[0:1, :], in_=rc_s[0:1, :],
                                 func=AF.Exp, scale=-1.0)
            nc.scalar.activation(out=rr_s[32:33, :], in_=rc_s[32:33, :],
                                 func=AF.Exp, scale=-1.0)

        bcs_tiles = {}

        def norm_bcast(m, t):
            # partition-broadcast 1/s on the PE: f32r selector matmul, then
            # stage to SBUF (DVE reads at most one PSUM operand per op)
            bc = o_ps.tile([128, 512], F32, name="o", tag="o", bufs=4)
            nc.tensor.matmul(bc, lhsT=sel_s[:], rhs=rr_s[:], start=True, stop=True)
            bcs = small.tile([128, 512], F32, name="bcs", tag="bcs", bufs=2)
            nc.vector.tensor_copy(out=bcs, in_=bc)
            bcs_tiles[(m, t)] = bcs

        def norm_apply(m, t):
            qsl = slice(t * 512, (t + 1) * 512)
            oA, oB = o_tiles.pop((m, t))
            bcs = bcs_tiles.pop((m, t))
            nc.vector.tensor_mul(ot_s[m][0:64, qsl], oA[0:64, :], bcs[0:64, :])
            nc.vector.tensor_mul(ot_s[m][64:128, qsl], oB[0:64, :], bcs[64:128, :])

        fo_tiles = {}

        def outproj_group(t2, mo, engine="vector", dma=None):
            if t2 not in fo_tiles:
                fo_tiles[t2] = small.tile([128, 4, 512], F32, name="fo",
                                          tag="fo", bufs=2)
            fo = fo_tiles[t2]
            ps = o_ps.tile([128, 512], F32, name="o", tag="o", bufs=4)
            for m in range(2):
                nc.tensor.matmul(
                    ps,
                    lhsT=wo_ap(m, mo),
                    rhs=ot_s[m][:, t2 * 512:(t2 + 1) * 512],
                    start=(m == 0), stop=(m == 1),
                )
            if engine == "vector":
                nc.vector.tensor_copy(out=fo[:, mo, :], in_=ps)
            else:
                nc.scalar.activation(out=fo[:, mo, :], in_=ps, func=AF.Copy)
            (dma or nc.gpsimd).dma_start(
                out=out_d[mo * 128:(mo + 1) * 128, t2 * 512:(t2 + 1) * 512],
                in_=fo[:, mo, :],
            )

        # ---------------- pipelined schedule ----------------
        # 64 global iterations (4 units x 16 kv tiles); scores emitted one
        # iteration ahead so ScalarE's exp stream never waits on PE.
        units = [(0, 0), (1, 0), (0, 1), (1, 1)]
        iters = [(u, i) for u in units for i in range(16)]

        # interleave remaining projections + V tiles + norms + out-proj
        # into the per-iteration PE slack (ACT exp is the steady-state pacer)
        extra = {g: [] for g in range(64)}
        kplan = [(0, 1), (0, 2), (0, 3), (1, 0), (1, 1), (1, 2), (1, 3)]
        for idx, (m_, t_) in enumerate(kplan):
            extra[2 * idx + 1].append(lambda m_=m_, t_=t_: kproj_group(m_, t_))
        extra[0].append(lambda: qproj_group(1, 0))
        for tt in range(16):
            extra[tt].append(lambda tt=tt: vproj_tile(tt))
        post = {
            17: [lambda: norm_recip(0, 0)],
            20: [lambda: norm_bcast(0, 0)],
            21: [lambda: norm_apply(0, 0)],
            25: [lambda: qproj_group(0, 1)],
            27: [lambda: qproj_group(1, 1)],
            33: [lambda: norm_recip(1, 0)],
            36: [lambda: norm_bcast(1, 0)],
            37: [lambda: norm_apply(1, 0)],
            41: [lambda: outproj_group(0, 0)],
            43: [lambda: outproj_group(0, 1)],
            45: [lambda: outproj_group(0, 2)],
            47: [lambda: outproj_group(0, 3)],
            49: [lambda: norm_recip(0, 1)],
            52: [lambda: norm_bcast(0, 1)],
            53: [lambda: norm_apply(0, 1)],
        }

        qproj_group(0, 0)
        kproj_group(0, 0)

        sc_tiles = {}

        def emit_scores(g):
            (m, t), i = iters[g]
            ksl = slice(i * 128, (i + 1) * 128)
            qsl = slice(t * 512, (t + 1) * 512)
            sc = sc_ps.tile([128, 2, 512], F32, name="sc", tag="sc")
            nc.tensor.matmul(
                sc[:, 0, :], lhsT=kt_s[m][0:64, ksl], rhs=qt_s[m][0:64, qsl],
                start=True, stop=True, tile_position=(0, 0),
            )
            nc.tensor.matmul(
                sc[:, 1, :], lhsT=kt_s[m][64:128, ksl], rhs=qt_s[m][64:128, qsl],
                start=True, stop=True, tile_position=(64, 0),
            )
            sc_tiles[g] = sc

        emit_scores(0)
        e_tiles = {}
        for g in range(64):
            (m, t), i = iters[g]
            u = g // 16
            if g + 1 < 64:
                emit_scores(g + 1)
            sc = sc_tiles.pop(g)
            e = esb.tile([128, 2, 512], BF16, name="e", tag="e", bufs=12)
            nc.scalar.activation(out=e[:], in_=sc[:], func=AF.Exp, scale=0.125)
            e_tiles[i] = e
            for fn in extra.get(g, ()):
                fn()
            # attn@V: unit 0 accumulates in step; later units defer their
            # first 8 kv tiles (so the previous unit's normalization can
            # release its PSUM accumulators) and catch up two tiles/iter.
            # ScalarE never starves: scores for g+1 are emitted above.
            if u == 0:
                todo = [i]
            elif i < 9:
                todo = []
            elif i < 15:
                todo = [2 * (i - 9), 2 * (i - 9) + 1]
            else:
                todo = [12, 13, 14, 15]
            if todo and (m, t) not in o_tiles:
                oA = o_ps.tile([65, 512], F32, name="o", tag="o", bufs=4)
                oB = o_ps.tile([65, 512], F32, name="o", tag="o", bufs=4)
                o_tiles[(m, t)] = (oA, oB)
            if todo:
                oA, oB = o_tiles[(m, t)]
                jA, jB = 2 * m, 2 * m + 1
                for j in todo:
                    ej = e_tiles[j]
                    nc.tensor.matmul(
                        oA, lhsT=v_s[:, j, jA, :], rhs=ej[:, 0, :],
                        start=(j == 0), stop=(j == 15),
                    )
                    nc.tensor.matmul(
                        oB, lhsT=v_s[:, j, jB, :], rhs=ej[:, 1, :],
                        start=(j == 0), stop=(j == 15),
                    )
            for fn in post.get(g, ()):
                fn()

        # ---------------- tail: last unit's norm + out-proj ----------------
        norm_recip(1, 1, tail=True)
        norm_bcast(1, 1)
        norm_apply(1, 1)
        outproj_group(1, 0, engine="scalar", dma=nc.sync)
        outproj_group(1, 1, engine="vector", dma=nc.scalar)
        outproj_group(1, 2, engine="scalar", dma=nc.gpsimd)
        outproj_group(1, 3, engine="vector", dma=nc.sync)

    _split_multi_waits(nc)
    return nc


_PROGRAM = None


def _get_program() -> bass.Bass:
    global _PROGRAM
    if _PROGRAM is None:
        _PROGRAM = _build_program()
    return _PROGRAM


def _prep_core_inputs(c, q, kv, Wqkv, bqkv, Wout):
    b, g = c // 2, c % 2
    cs = slice(256 * g, 256 * g + 256)
    wq = Wqkv[cs, :].T  # [512, 256]
    wk = Wqkv[512 + 256 * g:512 + 256 * g + 256, :].T
    wv = Wqkv[1024 + 256 * g:1024 + 256 * g + 256, :].T
    wo = Wout[:, cs].T  # [256, 512]

    # pack weights: [128, 4, 1024] with per-k-chunk [wq 256|wk 256|wv 256|wo 256]
    wb = np.empty((128, 4, 1024), np.float32)
    for k in range(4):
        rs = slice(128 * k, 128 * (k + 1))
        wb[:, k, 0:256] = wq[rs, :]
        wb[:, k, 256:512] = wk[rs, :]
        wb[:, k, 512:768] = wv[rs, :]
        wb[:, k, 768:1024] = wo.reshape(2, 128, 512).transpose(1, 0, 2).reshape(
            128, 1024)[:, 256 * k:256 * (k + 1)]
    bb = np.empty((128, 4), np.float32)
    bb[:, 0:2] = bqkv[cs].reshape(2, 128).T
    bb[:, 2:4] = bqkv[512 + 256 * g:512 + 256 * g + 256].reshape(2, 128).T
    return {
        "qx": np.ascontiguousarray(q[b].reshape(512, 1024)).astype(NP_BF16),
        "kvx": np.ascontiguousarray(kv[b].reshape(512, 2048)).astype(NP_BF16),
        "wb": np.ascontiguousarray(wb.reshape(128, 4096)).astype(NP_BF16),
        "bb": np.ascontiguousarray(bb),
    }


def kernel(q, kv, Wqkv, bqkv, Wout, bout):
    q = np.asarray(q, np.float32)
    kv = np.asarray(kv, np.float32)
    Wqkv = np.asarray(Wqkv, np.float32)
    bqkv = np.asarray(bqkv, np.float32)
    Wout = np.asarray(Wout, np.float32)
    bout = np.asarray(bout, np.float32)

    nc = _get_program()
    in_maps = [_prep_core_inputs(c, q, kv, Wqkv, bqkv, Wout) for c in range(8)]
    res = run_bass_kernel_spmd(nc, in_maps, list(range(8))).results

    # V-bias folds through softmax (rows sum to 1): bout' = bout + Wout @ bv
    bout_adj = bout + Wout @ bqkv[1024:1536]
    out = np.empty((4, 512, 32, 32), np.float32)
    for b in range(4):
        o = res[2 * b]["out"] + res[2 * b + 1]["out"] + bout_adj[:, None]
        out[b] = o.reshape(512, 32, 32)
    return out
